# revision 46
# baseline (speedup 1.0000x reference)
"""VQ codebook squared-distance kernel for Trainium2 (8 NeuronCores).

Computes dist[n,k,l] = (||x[n,:,l]||^2 + ||w[k,:]||^2 - 2*x[n,:,l].w[k,:]) / scale^2
for x (32,128,3136) f32, weight (64,128) f32, scale (1,) f32 -> out (32,64,3136) f32.

Sharding: data-parallel over N (4 per core); weight/scale replicated.

Design (best-measured configuration, 37.04us; nine HW iterations):
  - Input stream is HBM-stack-roofline-bound (~343 GB/s/core with both
    NCs of a stack active): 6.42 MB f32 x read in ~18us. The kernel is
    a saturated multi-resource equilibrium: PE ~14-16us, ACT ~14us,
    DVE ~13us busy inside a ~22us window, plus a FIXED ~8.7us NEFF
    end block (barrier + 257 walrus per-semaphore clears, invariant
    to kernel content and not HAM-gated).
  - x loads via SWDGE Q0 cast-on-load f32->f16 in graded pieces:
    small head (PE starts by ~11us), fine interleaved tail so each
    completion sem gates at most two chunks of matmuls. NOTE the
    SWDGE straggler: one SDMA engine lags the other 15 by an amount
    that grows with Q0 descriptor pressure (~0 at 8 transfers,
    ~1.65us at 12, ~2.4-4us beyond or with 3+ full-image transfers).
  - Chunks 6-7 of n2/n3 arrive via HWDGE (raw f32, ACT-cast to f16,
    DVE-squared straight from f32): lag-free sems, ready mid-stream.
    Never put the casts on GpSimd - Pool tensor ops lock the shared
    DVE/GpSimd SBUF ports and knock DVE out of 2x perf mode.
  - Outputs ride HWDGE in readiness-ordered pieces (no Q0 descriptor
    traffic, no queueing behind the straggler): pair 0 as one full-L
    write, pair 1 as ch(6,8)/ch(0,3)/ch(3,5)/ch(5,6) with the final
    50 KB piece last; late epilogues split ACT || DVE.
  - Output is offset fp8: e4m3(dist - 2D/s^2), host adds the offset
    back. Centering removes the ~2D/s^2 common mode so e4m3's ~6%
    step applies to the +-170 residual only: rel_l2 ~3e-3 (vs 2e-2
    budget) for half the write traffic.
  - PE: psum = (-2Wt)f16 @ x_f16 + ones_f16 @ (x^2)_f16, two images
    per PSUM tile via column tiling (tile_position (0,0)/(0,64)).
  - scale broadcast 1->128 via 1-col fp32 matmul; weight transpose on
    PE (identity built early on gpsimd).
"""

import numpy as np

N, D, L, K = 32, 128, 3136, 64
N_CORES = 8
NS = N // N_CORES          # n's per core
LC = 392                   # matmul chunk (8 per image, one PSUM bank)
HC = 196                   # half-chunk for the split tail epilogues

_cache = {}


def _build():
    import concourse.bacc as bacc
    import concourse.mybir as mybir
    import concourse.tile as tile
    from concourse.masks import make_identity

    f32 = mybir.dt.float32
    f16 = mybir.dt.float16
    f8 = mybir.dt.float8e4
    AF = mybir.ActivationFunctionType
    ALU = mybir.AluOpType

    nc = bacc.Bacc(
        "TRN2",
        target_bir_lowering=False,
        debug=False,
        enable_asserts=False,
        num_devices=N_CORES,
    )

    x_ap = nc.dram_tensor("x", (NS, D, L), f32, kind="ExternalInput").ap()
    w_ap = nc.dram_tensor("weight", (K, D), f32, kind="ExternalInput").ap()
    s_ap = nc.dram_tensor("scale", (1,), f32, kind="ExternalInput").ap()
    o_ap = nc.dram_tensor("out", (NS, K, L), f8, kind="ExternalOutput").ap()

    def ch(a, b):  # cols covering chunks [a, b)
        return slice(a * LC, b * LC)

    # Q0 (SWDGE cast-on-load) transfer plan: graded sizes so PE starts
    # early and is then fed continuously; fine-grained interleaved
    # pieces for the second pair so each completion sem gates at most
    # two chunks of matmuls.
    stream = [
        (0, ch(0, 2)), (1, ch(0, 2)),
        (0, ch(2, 5)), (1, ch(2, 5)),
        (0, ch(5, 8)), (1, ch(5, 8)),
        (2, ch(0, 3)), (3, ch(0, 3)),
        (2, ch(3, 4)), (3, ch(3, 4)),
        (2, ch(4, 5)), (3, ch(4, 5)),
    ]

    with tile.TileContext(nc) as tc:
        with (
            tc.tile_pool(name="consts", bufs=1) as consts,
            tc.tile_pool(name="xin", bufs=4) as xpool,
            tc.tile_pool(name="xsq", bufs=4) as xqpool,
            tc.tile_pool(name="outp", bufs=2) as opool,
            tc.tile_pool(name="psum", bufs=4, space="PSUM") as pspool,
            tc.tile_pool(name="psum1", bufs=1, space="PSUM") as pspool1,
        ):
            xts = [
                xpool.tile([D, L], f16, tag="xt", name=f"x_{n}")
                for n in range(NS)
            ]
            xqs = [
                xqpool.tile([D, L], f16, tag="xq", name=f"xsq_{n}")
                for n in range(NS)
            ]

            # ---- input stream (SWDGE Q0, cast f32->f16 on load) ----------
            ident = consts.tile([K, K], f32)
            for i, (n, sl) in enumerate(stream):
                nc.gpsimd.dma_start(out=xts[n][:, sl], in_=x_ap[n][:, sl])
                if i == 0:
                    make_identity(nc, ident)

            # ---- HWDGE raw-f32 loads for chunks 5-7 of n2/n3 -------------
            # These drain alongside the Q0 stream and land mid-stream with
            # negligible completion lag; DVE casts them to f16 into a
            # separate single-writer tile xc (ACT is the back-half
            # bottleneck at ~480ns fixed cost per instruction — keep it
            # free for epilogues only; a shared-writer tile would get
            # cross-engine write ordering), then squares the f16 at 2x.
            xfs, xcs = {}, {}
            for n in (2, 3):
                xf = xpool.tile([D, 3 * LC], f32, tag="xf", name=f"xf_{n}")
                xfs[n] = xf
                nc.sync.dma_start(out=xf, in_=x_ap[n][:, ch(5, 8)])
                xcs[n] = xpool.tile([D, 3 * LC], f16, tag="xc", name=f"xc_{n}")

            # ---- weight / scale prep (HWDGE, overlaps the stream) --------
            s_t = consts.tile([1, 1], f32)
            nc.sync.dma_start(out=s_t, in_=s_ap.to_broadcast((1, 1)))
            w2 = consts.tile([2 * K, D], f32)
            nc.sync.dma_start(out=w2[0:K, :], in_=w_ap)
            nc.sync.dma_start(out=w2[K : 2 * K, :], in_=w_ap)

            ones_row = consts.tile([1, 128], f32)
            nc.vector.memset(ones_row, 1.0)
            ones16 = consts.tile([D, K], f16)
            nc.vector.memset(ones16, 1.0)

            # broadcast scale to all 128 partitions via 1-col fp32 matmul
            ps_s = pspool1.tile([128, 1], f32, name="ps_s")
            nc.tensor.matmul(ps_s, ones_row, s_t, start=True, stop=True)
            s_b = consts.tile([128, 1], f32)
            nc.vector.tensor_scalar_mul(s_b, in0=ps_s, scalar1=1.0)
            inv_s2 = consts.tile([128, 1], f32)
            nc.vector.tensor_mul(inv_s2, s_b, s_b)
            nc.vector.reciprocal(inv_s2, inv_s2)

            w_sq = consts.tile([2 * K, D], f32)
            nc.vector.tensor_mul(w_sq, w2, w2)
            c_sq = consts.tile([2 * K, 1], f32)
            nc.vector.reduce_sum(out=c_sq, in_=w_sq, axis=mybir.AxisListType.X)
            c_sq_s = consts.tile([2 * K, 1], f32)
            nc.vector.tensor_mul(c_sq_s, c_sq, inv_s2)
            # fp8 offset encoding: store e4m3(dist - 2D/s^2); the host adds
            # the offset back. Centering kills the common mode so e4m3's
            # 6% relative step lands on the +-170 residual.
            bias2 = consts.tile([2 * K, 1], f32)
            nc.vector.tensor_scalar(
                out=bias2, in0=inv_s2,
                scalar1=-float(2 * D), scalar2=c_sq_s,
                op0=ALU.mult, op1=ALU.add,
            )

            ps_w = pspool1.tile([D, K], f32, name="ps_w")
            nc.tensor.transpose(ps_w, w2[0:K, :], ident)
            wT16 = consts.tile([D, K], f16)
            nc.vector.tensor_scalar_mul(wT16, in0=ps_w, scalar1=-2.0)

            # ---- derived stream: casts + fp16 x^2 on DVE, arrival order --
            for n, sl in stream[:5]:
                nc.vector.tensor_mul(xqs[n][:, sl], xts[n][:, sl], xts[n][:, sl])
            for n in (2, 3):
                nc.vector.tensor_scalar_mul(xcs[n], in0=xfs[n], scalar1=1.0)
            n, sl = stream[5]
            nc.vector.tensor_mul(xqs[n][:, sl], xts[n][:, sl], xts[n][:, sl])
            for n in (2, 3):
                nc.vector.tensor_mul(xqs[n][:, ch(5, 8)], xcs[n], xcs[n])
            for n, sl in stream[6:]:
                nc.vector.tensor_mul(xqs[n][:, sl], xts[n][:, sl], xts[n][:, sl])

            # ---- matmuls + epilogues + HWDGE output pieces ---------------
            def mm_quad(ps, n0, n1, c):
                sl = ch(c, c + 1)
                nc.tensor.matmul(
                    ps[0:K, :], wT16, xts[n0][:, sl],
                    start=True, stop=False, tile_position=(0, 0),
                )
                nc.tensor.matmul(
                    ps[K : 2 * K, :], wT16, xts[n1][:, sl],
                    start=True, stop=False, tile_position=(0, 64),
                )
                nc.tensor.matmul(
                    ps[0:K, :], ones16, xqs[n0][:, sl],
                    start=False, stop=True, tile_position=(0, 0),
                )
                nc.tensor.matmul(
                    ps[K : 2 * K, :], ones16, xqs[n1][:, sl],
                    start=False, stop=True, tile_position=(0, 64),
                )

            # pair 0: natural chunk order, paced by the stream
            out_t0 = opool.tile([2 * K, L], f8, tag="out_t", name="out_0")
            o_p0 = o_ap[0:2].rearrange("a k l -> (a k) l")
            for c in range(8):
                ps = pspool.tile([2 * K, LC], f32, name="ps")
                mm_quad(ps, 0, 1, c)
                nc.scalar.activation(
                    out_t0[:, ch(c, c + 1)], ps, AF.Identity,
                    bias=bias2, scale=inv_s2,
                )
            nc.sync.dma_start(out=o_p0, in_=out_t0)

            # pair 1: HWDGE-fed chunks 5-7 first (ready mid-stream), then
            # the Q0-fed chunks in arrival order. The tail chunks c3/c4
            # get FULL-DVE epilogues into a separate single-writer tile:
            # a DVE write into out_t1 would be serialized behind ACT's
            # epilogues by the tile-level cross-engine write ordering
            # (measured: the DVE half waited on the ACT half's sem).
            out_t1 = opool.tile([2 * K, L], f8, tag="out_t", name="out_1")
            out_tl = opool.tile([2 * K, 2 * LC], f8, tag="out_tl", name="out_tl")
            o_p1 = o_ap[2:4].rearrange("a k l -> (a k) l")
            for c in (5, 6, 7):
                ps = pspool.tile([2 * K, LC], f32, name="ps")
                fsl = slice((c - 5) * LC, (c - 4) * LC)
                sl = ch(c, c + 1)
                nc.tensor.matmul(
                    ps[0:K, :], wT16, xcs[2][:, fsl],
                    start=True, stop=False, tile_position=(0, 0),
                )
                nc.tensor.matmul(
                    ps[K : 2 * K, :], wT16, xcs[3][:, fsl],
                    start=True, stop=False, tile_position=(0, 64),
                )
                nc.tensor.matmul(
                    ps[0:K, :], ones16, xqs[2][:, sl],
                    start=False, stop=True, tile_position=(0, 0),
                )
                nc.tensor.matmul(
                    ps[K : 2 * K, :], ones16, xqs[3][:, sl],
                    start=False, stop=True, tile_position=(0, 64),
                )
                nc.scalar.activation(
                    out_t1[:, sl], ps, AF.Identity,
                    bias=bias2, scale=inv_s2,
                )
            es = ch(5, 8)
            nc.sync.dma_start(out=o_p1[:, es], in_=out_t1[:, es])
            for c in (0, 1, 2):
                ps = pspool.tile([2 * K, LC], f32, name="ps")
                mm_quad(ps, 2, 3, c)
                nc.scalar.activation(
                    out_t1[:, ch(c, c + 1)], ps, AF.Identity,
                    bias=bias2, scale=inv_s2,
                )
            hs = ch(0, 3)
            nc.sync.dma_start(out=o_p1[:, hs], in_=out_t1[:, hs])
            for c in (3, 4):
                ps = pspool.tile([2 * K, LC], f32, name="ps")
                mm_quad(ps, 2, 3, c)
                nc.vector.tensor_scalar(
                    out=out_tl[:, (c - 3) * LC : (c - 2) * LC],
                    in0=ps,
                    scalar1=inv_s2, scalar2=bias2,
                    op0=ALU.mult, op1=ALU.add,
                )
            qs = ch(3, 5)
            nc.sync.dma_start(out=o_p1[:, qs], in_=out_tl)

    nc.compile()
    return nc


def _get_nc():
    if "nc" not in _cache:
        _cache["nc"] = _build()
    return _cache["nc"]


def run(x, weight, scale, trace=False, tmpdir=None):
    from concourse.bass_utils import run_bass_kernel_spmd

    x = np.ascontiguousarray(np.asarray(x, dtype=np.float32))
    weight = np.ascontiguousarray(np.asarray(weight, dtype=np.float32))
    scale = np.ascontiguousarray(np.asarray(scale, dtype=np.float32))
    assert x.shape == (N, D, L) and weight.shape == (K, D) and scale.shape == (1,)

    nc = _get_nc()
    in_maps = [
        {"x": x[c * NS : (c + 1) * NS], "weight": weight, "scale": scale}
        for c in range(N_CORES)
    ]
    res = run_bass_kernel_spmd(
        nc, in_maps, core_ids=list(range(N_CORES)), trace=trace, tmpdir=tmpdir
    )
    out = np.concatenate([r["out"] for r in res.results], axis=0).astype(np.float32)
    out += np.float32(2.0 * D) / np.float32(scale[0] ** 2)
    return out, res


def kernel(x, weight, scale):
    out, _ = run(x, weight, scale, trace=False)
    return out
